# revision 11
# baseline (speedup 1.0000x reference)
"""Multi-head attention (B=2,S=2048,D=512,H=8,depth=64) + causal-mask softmax
+ output projection + residual + LayerNorm, returning (out, attn).

Sharding: sequence-parallel over query rows. 8 cores; core c handles
batch b = c // 4 and query rows [ (c%4)*512 : (c%4+1)*512 ).
Each core computes its full slice locally (no collectives): K/V projections
are recomputed per batch-group of 4 cores (cheap), attention/softmax/output
are fully local to the core's query rows.

Device kernel layout strategy (per core):
  - host uploads qT [D,512], q native [512,D], kT [D,2048], vT [D,2048],
    mask slice [512,2048], weights/biases/ln params.
  - Q^T = wq^T @ qT (+bq)    -> [D, 512]   (per-head lhsT for logits)
  - K^T = wk^T @ kT (+bk)    -> [D, 2048]  (per-head rhs for logits)
  - V   = vT_tiles^T @ wv (+bv) -> [2048, D] (per-head lhsT for PV)
  - logits psum = Q_h^T.T @ K_h^T  accumulated with (-80000*mask) via
    identity-matmul; exp on ScalarE with scale=1/8 and accum_out row-sums
  - P = exp * (1/rowsum)  (DVE tensor_scalar, in-place) -> attn output DMA
  - P^T via PE transposes -> PV matmul -> ctx^T [64, q] per head
  - O = ctx^T.T @ wo (+bo) + residual, LayerNorm via bn_stats -> out DMA
"""

import numpy as np

import concourse.bass as bass
import concourse.bacc as bacc
import concourse.tile as tile
from concourse import mybir
from concourse.masks import make_identity
from concourse.bass_utils import run_bass_kernel_spmd

B, S, D, H, DEPTH = 2, 2048, 512, 8, 64
QB = 512               # query rows per core
NQT = QB // 128        # q-tiles per core
NCORES = 8
FP = mybir.dt.float32
MASK_SCALE = -80000.0  # -10000 * 8; exp applies scale=1/8
EPS = 1e-6

TRACE = False          # test.py sets this for profiled runs


def _build_nc():
    # Bacc (not raw Bass): its compile() pipeline runs
    # move_matmul_waits_to_ldweights + generate_event_semaphores, which
    # legalize to the TRN2 "at most 1 sync-wait per instruction" constraint
    # that this walrus build enforces.
    nc = bacc.Bacc(None, target_bir_lowering=False)

    # ---- I/O ----
    qT_d = nc.dram_tensor("qT", [D, QB], FP, kind="ExternalInput")
    qn_d = nc.dram_tensor("qn", [QB, D], FP, kind="ExternalInput")
    kT_d = nc.dram_tensor("kT", [D, S], FP, kind="ExternalInput")
    vT_d = nc.dram_tensor("vT", [D, S], FP, kind="ExternalInput")
    mk_d = nc.dram_tensor("mk", [QB, S], FP, kind="ExternalInput")
    wq_d = nc.dram_tensor("wq", [D, D], FP, kind="ExternalInput")
    wk_d = nc.dram_tensor("wk", [D, D], FP, kind="ExternalInput")
    wv_d = nc.dram_tensor("wv", [D, D], FP, kind="ExternalInput")
    wo_d = nc.dram_tensor("wo", [D, D], FP, kind="ExternalInput")
    bq_d = nc.dram_tensor("bq", [D], FP, kind="ExternalInput")
    bk_d = nc.dram_tensor("bk", [D], FP, kind="ExternalInput")
    bv_d = nc.dram_tensor("bv", [D], FP, kind="ExternalInput")
    bo_d = nc.dram_tensor("bo", [D], FP, kind="ExternalInput")
    ga_d = nc.dram_tensor("ga", [D], FP, kind="ExternalInput")
    be_d = nc.dram_tensor("be", [D], FP, kind="ExternalInput")
    attn_d = nc.dram_tensor("attn_o", [H, QB, S], FP, kind="ExternalOutput")
    out_d = nc.dram_tensor("out_o", [QB, D], FP, kind="ExternalOutput")

    with tile.TileContext(nc) as tc:
        _emit(nc, tc, qT_d, qn_d, kT_d, vT_d, mk_d,
              wq_d, wk_d, wv_d, wo_d, bq_d, bk_d, bv_d, bo_d,
              ga_d, be_d, attn_d, out_d)
    nc.compile()
    return nc


def _emit(nc, tc, qT_d, qn_d, kT_d, vT_d, mk_d,
          wq_d, wk_d, wv_d, wo_d, bq_d, bk_d, bv_d, bo_d,
          ga_d, be_d, attn_d, out_d):
    from contextlib import ExitStack

    es = ExitStack()
    with es:
        consts = es.enter_context(tc.tile_pool(name="consts", bufs=1))
        persist = es.enter_context(tc.tile_pool(name="persist", bufs=1))
        wo_pool = es.enter_context(tc.tile_pool(name="wo", bufs=1))

        # ---- constants ----
        ident = consts.tile([128, 128], FP)
        make_identity(nc, ident)
        i80k = consts.tile([128, 128], FP)
        make_identity(nc, i80k)
        nc.vector.tensor_scalar_mul(i80k, i80k, MASK_SCALE)
        ones = consts.tile([1, QB], FP)
        nc.vector.memset(ones, 1.0)
        eps_t = consts.tile([128, 1], FP)
        nc.vector.memset(eps_t, EPS)
        bq_s = consts.tile([1, D], FP)
        bk_s = consts.tile([1, D], FP)
        bv_s = consts.tile([1, D], FP)
        bo_s = consts.tile([1, D], FP)
        nc.sync.dma_start(out=bq_s, in_=bq_d[None, :])
        nc.sync.dma_start(out=bk_s, in_=bk_d[None, :])
        nc.sync.dma_start(out=bv_s, in_=bv_d[None, :])
        nc.sync.dma_start(out=bo_s, in_=bo_d[None, :])
        ga_b = consts.tile([128, D], FP)
        be_b = consts.tile([128, D], FP)
        nc.sync.dma_start(out=ga_b, in_=ga_d[None, :].to_broadcast([128, D]))
        nc.sync.dma_start(out=be_b, in_=be_d[None, :].to_broadcast([128, D]))

        # ---- persistent SBUF ----
        KT = [persist.tile([128, S], FP, tag=f"KT{i}", name=f"KT{i}") for i in range(4)]
        Vt = [persist.tile([128, D], FP, tag=f"V{i}", name=f"V{i}") for i in range(16)]
        QT = [persist.tile([128, QB], FP, tag=f"QT{i}", name=f"QT{i}") for i in range(4)]
        MB = [persist.tile([128, S], FP, tag=f"MB{i}", name=f"MB{i}") for i in range(4)]
        QN = [persist.tile([128, D], FP, tag=f"QN{i}", name=f"QN{i}") for i in range(4)]
        CT = [persist.tile([128, QB], FP, tag=f"CT{i}", name=f"CT{i}") for i in range(4)]
        wo_s = [wo_pool.tile([128, D], FP, tag=f"wo{i}", name=f"wos{i}") for i in range(4)]

        wo_r = wo_d[:].rearrange("(t p) d -> t p d", p=128)
        for i in range(4):
            nc.sync.dma_start(out=wo_s[i], in_=wo_r[i])
            nc.sync.dma_start(out=MB[i], in_=mk_d[i * 128:(i + 1) * 128, :])
            nc.sync.dma_start(out=QN[i], in_=qn_d[i * 128:(i + 1) * 128, :])

        # ---- phase A: projections (scoped pools) ----
        with (
            tc.tile_pool(name="wqkv", bufs=1) as wqkv,
            tc.tile_pool(name="instream", bufs=4) as instream,
            tc.tile_pool(name="pA", bufs=6, space="PSUM") as pA,
        ):
            wq_s = [wqkv.tile([128, D], FP, tag=f"wq{i}", name=f"wqs{i}") for i in range(4)]
            wk_s = [wqkv.tile([128, D], FP, tag=f"wk{i}", name=f"wks{i}") for i in range(4)]
            wv_s = [wqkv.tile([128, D], FP, tag=f"wv{i}", name=f"wvs{i}") for i in range(4)]
            wq_r = wq_d[:].rearrange("(t p) d -> t p d", p=128)
            wk_r = wk_d[:].rearrange("(t p) d -> t p d", p=128)
            wv_r = wv_d[:].rearrange("(t p) d -> t p d", p=128)
            for i in range(4):
                nc.sync.dma_start(out=wq_s[i], in_=wq_r[i])
                nc.sync.dma_start(out=wk_s[i], in_=wk_r[i])
                nc.sync.dma_start(out=wv_s[i], in_=wv_r[i])

            # Q^T projection: psum[dout] = sum_din wq[din,dout]^T-slice @ qT
            qtin = [instream.tile([128, QB], FP, tag="qtin", name="qtin") for _ in range(4)]
            for din in range(4):
                nc.sync.dma_start(
                    out=qtin[din], in_=qT_d[din * 128:(din + 1) * 128, :])
            ps_q = [pA.tile([128, QB], FP, tag="pa", name="psq") for _ in range(4)]
            for dout in range(4):
                for din in range(4):
                    nc.tensor.matmul(
                        ps_q[dout], wq_s[din][:, dout * 128:(dout + 1) * 128],
                        qtin[din], start=(din == 0), stop=False)
                nc.tensor.matmul(
                    ps_q[dout], bq_s[:, dout * 128:(dout + 1) * 128], ones,
                    start=False, stop=True)
                nc.vector.tensor_copy(QT[dout], ps_q[dout])

            # K^T projection, sk in chunks of 512
            for skc in range(4):
                ktin = [instream.tile([128, 512], FP, tag="ktin", name="ktin", bufs=6)
                        for _ in range(4)]
                for din in range(4):
                    nc.sync.dma_start(
                        out=ktin[din],
                        in_=kT_d[din * 128:(din + 1) * 128,
                                 skc * 512:(skc + 1) * 512])
                ps_k = [pA.tile([128, 512], FP, tag="pa", name="psk") for _ in range(4)]
                for dout in range(4):
                    for din in range(4):
                        nc.tensor.matmul(
                            ps_k[dout],
                            wk_s[din][:, dout * 128:(dout + 1) * 128],
                            ktin[din], start=(din == 0), stop=False)
                    nc.tensor.matmul(
                        ps_k[dout], bk_s[:, dout * 128:(dout + 1) * 128],
                        ones[:, :512], start=False, stop=True)
                    dst = KT[dout][:, skc * 512:(skc + 1) * 512]
                    if dout % 2 == 0:
                        nc.vector.tensor_copy(dst, ps_k[dout])
                    else:
                        nc.scalar.copy(dst, ps_k[dout])

            # V projection: psum[sk-tile] = sum_din vT[din, sk]^T-slice @ wv
            for skc in range(4):
                vtin = [instream.tile([128, 512], FP, tag="vtin", name="vtin", bufs=6)
                        for _ in range(4)]
                for din in range(4):
                    nc.sync.dma_start(
                        out=vtin[din],
                        in_=vT_d[din * 128:(din + 1) * 128,
                                 skc * 512:(skc + 1) * 512])
                for st in range(4):
                    sk = skc * 4 + st
                    ps_v = pA.tile([128, D], FP, tag="pa")
                    for din in range(4):
                        nc.tensor.matmul(
                            ps_v, vtin[din][:, st * 128:(st + 1) * 128],
                            wv_s[din], start=(din == 0), stop=False)
                    nc.tensor.matmul(
                        ps_v, ones[:, :128], bv_s, start=False, stop=True)
                    if st % 2 == 0:
                        nc.vector.tensor_copy(Vt[sk], ps_v)
                    else:
                        nc.scalar.copy(Vt[sk], ps_v)

        # ---- hot phase pools ----
        with (
            tc.tile_pool(name="Pp", bufs=3) as Pp,
            tc.tile_pool(name="PTp", bufs=2) as PTp,
            tc.tile_pool(name="sp", bufs=8) as sp,
            tc.tile_pool(name="Osb", bufs=2) as Osb,
            tc.tile_pool(name="Lp", bufs=2, space="PSUM") as Lp,
            tc.tile_pool(name="Tp", bufs=2, space="PSUM") as Tp,
            tc.tile_pool(name="Cp", bufs=2, space="PSUM") as Cp,
        ):
            for h in range(8):
                ht = h // 2
                hp = (h % 2) * 64
                PT_t = None
                for qt in range(4):
                    if qt % 2 == 0:
                        PT_t = PTp.tile([128, 4096], FP, tag="pt")
                    # logits + mask into psum, two 1024-halves
                    P_t = Pp.tile([128, S], FP, tag="p")
                    s_half = []
                    for half in range(2):
                        L_t = Lp.tile([128, 1024], FP, tag="L")
                        for sc in range(2):
                            skc = half * 2 + sc
                            sl = slice(sc * 512, (sc + 1) * 512)
                            nc.tensor.matmul(
                                L_t[:, sl],
                                QT[ht][hp:hp + 64, qt * 128:(qt + 1) * 128],
                                KT[ht][hp:hp + 64,
                                       skc * 512:(skc + 1) * 512],
                                start=True, stop=False)
                            nc.tensor.matmul(
                                L_t[:, sl], i80k,
                                MB[qt][:, skc * 512:(skc + 1) * 512],
                                start=False, stop=True)
                        sh = sp.tile([128, 1], FP, tag="sh")
                        nc.scalar.activation(
                            out=P_t[:, half * 1024:(half + 1) * 1024],
                            in_=L_t, func=mybir.ActivationFunctionType.Exp,
                            scale=0.125, accum_out=sh)
                        s_half.append(sh)
                    rs = sp.tile([128, 1], FP, tag="rs")
                    nc.vector.tensor_add(rs, s_half[0], s_half[1])
                    nc.vector.reciprocal(rs, rs)
                    nc.vector.tensor_scalar_mul(P_t, P_t, rs)
                    nc.sync.dma_start(
                        out=attn_d[h, qt * 128:(qt + 1) * 128, :], in_=P_t)
                    # transpose 16x [128,128] -> PT
                    PT_v = PT_t.rearrange("p (j two q) -> p j two q",
                                          two=2, q=128)
                    for jg in range(4):
                        T_t = Tp.tile([128, 512], FP, tag="tp")
                        for jj in range(4):
                            j = jg * 4 + jj
                            nc.tensor.transpose(
                                T_t[:, jj * 128:(jj + 1) * 128],
                                P_t[:, j * 128:(j + 1) * 128], ident)
                        dst = PT_v[:, jg * 4:(jg + 1) * 4, qt % 2, :]
                        if jg % 2 == 0:
                            nc.vector.tensor_copy(dst, T_t)
                        else:
                            nc.scalar.copy(dst, T_t)
                    if qt % 2 == 1:
                        pair = qt // 2
                        ct = Cp.tile([64, 256], FP, tag="ctx")
                        for j in range(16):
                            nc.tensor.matmul(
                                ct, Vt[j][:, h * 64:(h + 1) * 64],
                                PT_t[:, j * 256:(j + 1) * 256],
                                start=(j == 0), stop=(j == 15))
                        nc.vector.tensor_copy(
                            CT[ht][hp:hp + 64,
                                   pair * 256:(pair + 1) * 256], ct)

            # ---- phase C: output projection + residual + LayerNorm ----
            for qt in range(4):
                O_ps = Tp.tile([128, 512], FP, tag="tp")
                for din in range(4):
                    nc.tensor.matmul(
                        O_ps, CT[din][:, qt * 128:(qt + 1) * 128],
                        wo_s[din], start=(din == 0), stop=False)
                nc.tensor.matmul(
                    O_ps, ones[:, :128], bo_s, start=False, stop=True)
                O_sb = Osb.tile([128, D], FP, tag="osb")
                nc.vector.tensor_add(O_sb, O_ps, QN[qt])
                stats = sp.tile([128, 6], FP, tag="st")
                mv = sp.tile([128, 2], FP, tag="mv")
                nc.vector.bn_stats(stats, O_sb)
                nc.vector.bn_aggr(mv, stats)
                rstd = sp.tile([128, 1], FP, tag="rstd")
                nc.scalar.activation(
                    out=rstd, in_=mv[:, 1:2],
                    func=mybir.ActivationFunctionType.Sqrt, bias=eps_t)
                nc.vector.reciprocal(rstd, rstd)
                nc.vector.tensor_scalar(
                    out=O_sb, in0=O_sb, scalar1=mv[:, 0:1], scalar2=rstd,
                    op0=mybir.AluOpType.subtract, op1=mybir.AluOpType.mult)
                nc.vector.tensor_mul(O_sb, O_sb, ga_b)
                nc.vector.tensor_add(O_sb, O_sb, be_b)
                nc.sync.dma_start(
                    out=out_d[qt * 128:(qt + 1) * 128, :], in_=O_sb)


_NC_CACHE = None


def _get_nc():
    global _NC_CACHE
    if _NC_CACHE is None:
        _NC_CACHE = _build_nc()
    return _NC_CACHE


def kernel(query, key, value, mask, wq, bq, wk, bk, wv, bv, wo, bo,
           gamma, beta):
    query = np.ascontiguousarray(np.asarray(query, np.float32))
    key = np.ascontiguousarray(np.asarray(key, np.float32))
    value = np.ascontiguousarray(np.asarray(value, np.float32))
    mask = np.ascontiguousarray(np.asarray(mask, np.float32))
    wq = np.ascontiguousarray(np.asarray(wq, np.float32))
    wk = np.ascontiguousarray(np.asarray(wk, np.float32))
    wv = np.ascontiguousarray(np.asarray(wv, np.float32))
    wo = np.ascontiguousarray(np.asarray(wo, np.float32))
    bq = np.asarray(bq, np.float32)
    bk = np.asarray(bk, np.float32)
    bv = np.asarray(bv, np.float32)
    bo = np.asarray(bo, np.float32)
    gamma = np.asarray(gamma, np.float32)
    beta = np.asarray(beta, np.float32)

    nc = _get_nc()
    in_maps = []
    for c in range(NCORES):
        b = c // 4
        q0 = (c % 4) * QB
        in_maps.append({
            "qT": np.ascontiguousarray(query[b, q0:q0 + QB, :].T),
            "qn": np.ascontiguousarray(query[b, q0:q0 + QB, :]),
            "kT": np.ascontiguousarray(key[b].T),
            "vT": np.ascontiguousarray(value[b].T),
            "mk": np.ascontiguousarray(mask[b, 0, q0:q0 + QB, :]),
            "wq": wq, "wk": wk, "wv": wv, "wo": wo,
            "bq": bq, "bk": bk, "bv": bv, "bo": bo,
            "ga": gamma, "be": beta,
        })

    r = run_bass_kernel_spmd(nc, in_maps, core_ids=list(range(NCORES)),
                             trace=TRACE)
    if TRACE:
        kernel.last_results = r

    out = np.empty((B, S, D), np.float32)
    attn = np.empty((B, H, S, S), np.float32)
    for c in range(NCORES):
        b = c // 4
        q0 = (c % 4) * QB
        out[b, q0:q0 + QB, :] = r.results[c]["out_o"]
        attn[b, :, q0:q0 + QB, :] = r.results[c]["attn_o"]
    return out, attn


# revision 13
# speedup vs baseline: 1.9002x; 1.9002x over previous
"""Multi-head attention (B=2,S=2048,D=512,H=8,depth=64) + causal-mask softmax
+ output projection + residual + LayerNorm, returning (out, attn).

Sharding: sequence-parallel over query rows, causal-load-balanced. 8 cores;
core c handles batch b = c // 4 and the four global 128-row q-tiles
{j, 7-j, 8+j, 15-j} (j = c % 4). With a causal mask, local q-tile t then
has exactly t+1 live 512-wide k-chunks on EVERY core, so the SPMD program
is uniform while skipping the fully-masked upper-triangular blocks.
A dense variant (all chunks live, additive mask everywhere) is built when
the host detects the mask is not exactly causal.

Per-core device pipeline:
  - host uploads qT [D,512](transposed q rows), qn (= q rows + bo), kT, vT,
    mask chunks (bf16), weights, biases, ln params.
  - Q^T = wq^T @ qT, K^T = wk^T @ kT (bias fused into the PSUM->SBUF copy
    as a per-partition tensor_scalar add), V = vT_tiles^T @ wv (bias via
    broadcast tensor_add in the copy).
  - logits psum = Q_h^T.T @ K_h^T (fp32) + bf16 identity-matmul of
    (-79872*mask) for masked chunks; exp on ScalarE (scale=1/8) with
    accum_out row-sums; reciprocal; normalize in-place (tensor_scalar).
  - attn output DMA (plus a zero-tile DMA for skipped chunks).
  - P^T via PE transposes; PV matmul -> normalized ctx^T [64, q] per head.
  - O = ctx^T.T @ wo + residual(qn) then LayerNorm via bn_stats -> out DMA.
"""

import numpy as np
import ml_dtypes

import concourse.bacc as bacc
import concourse.tile as tile
from concourse import mybir
from concourse.masks import make_identity
from concourse.bass_utils import run_bass_kernel_spmd

B, S, D, H, DEPTH = 2, 2048, 512, 8, 64
QB = 512               # query rows per core
NQT = QB // 128        # local q-tiles per core
NCORES = 8
FP = mybir.dt.float32
BF = mybir.dt.bfloat16
MASK_SCALE = -80000.0  # ~ -10000 * 8 (exp applies scale=1/8); bf16-rounded
EPS = 1e-6

TRACE = False          # test.py sets this for profiled runs


def _build_nc(causal: bool):
    nc = bacc.Bacc(None, target_bir_lowering=False)

    # ---- I/O ----
    d = {}
    d["qT"] = nc.dram_tensor("qT", [D, QB], FP, kind="ExternalInput")
    d["qn"] = nc.dram_tensor("qn", [QB, D], FP, kind="ExternalInput")
    d["kT"] = nc.dram_tensor("kT", [D, S], FP, kind="ExternalInput")
    d["vT"] = nc.dram_tensor("vT", [D, S], FP, kind="ExternalInput")
    if causal:
        # per local q-tile: only the diagonal 512-chunk of the mask
        d["mk"] = nc.dram_tensor("mk", [NQT, 128, 512], BF,
                                 kind="ExternalInput")
    else:
        d["mk"] = nc.dram_tensor("mk", [QB, S], BF, kind="ExternalInput")
    for w in ("wq", "wk", "wv", "wo"):
        d[w] = nc.dram_tensor(w, [D, D], FP, kind="ExternalInput")
    for bnm in ("bq", "bk", "bv", "ga", "be"):
        d[bnm] = nc.dram_tensor(bnm, [D], FP, kind="ExternalInput")
    d["attn_o"] = nc.dram_tensor("attn_o", [H, QB, S], FP,
                                 kind="ExternalOutput")
    d["out_o"] = nc.dram_tensor("out_o", [QB, D], FP, kind="ExternalOutput")

    with tile.TileContext(nc) as tc:
        _emit(nc, tc, d, causal)
    nc.compile()
    return nc


def _emit(nc, tc, d, causal):
    from contextlib import ExitStack

    es = ExitStack()
    with es:
        consts = es.enter_context(tc.tile_pool(name="consts", bufs=1))
        persist = es.enter_context(tc.tile_pool(name="persist", bufs=1))
        wo_pool = es.enter_context(tc.tile_pool(name="wo", bufs=1))

        # ---- constants ----
        ident = consts.tile([128, 128], FP)
        make_identity(nc, ident)
        i80k = consts.tile([128, 128], BF)
        make_identity(nc, i80k)
        nc.vector.tensor_scalar_mul(i80k, i80k, MASK_SCALE)
        eps_t = consts.tile([128, 1], FP)
        nc.vector.memset(eps_t, EPS)
        # bias columns [128, 4]: bq/bk rearranged so tile t's per-partition
        # bias column is bqc[:, t]
        bqc = consts.tile([128, 4], FP)
        bkc = consts.tile([128, 4], FP)
        nc.sync.dma_start(out=bqc, in_=d["bq"][:].rearrange("(t p) -> p t",
                                                            p=128))
        nc.sync.dma_start(out=bkc, in_=d["bk"][:].rearrange("(t p) -> p t",
                                                            p=128))
        bv_b = consts.tile([128, D], FP)
        nc.sync.dma_start(out=bv_b, in_=d["bv"][None, :].to_broadcast(
            [128, D]))
        ga_b = consts.tile([128, D], FP)
        be_b = consts.tile([128, D], FP)
        nc.sync.dma_start(out=ga_b, in_=d["ga"][None, :].to_broadcast(
            [128, D]))
        nc.sync.dma_start(out=be_b, in_=d["be"][None, :].to_broadcast(
            [128, D]))
        if causal:
            zerot = consts.tile([128, 1536], FP)
            nc.vector.memset(zerot, 0.0)

        # ---- persistent SBUF ----
        KT = [persist.tile([128, S], FP, tag=f"KT{i}", name=f"KT{i}")
              for i in range(4)]
        Vt = [persist.tile([128, D], FP, tag=f"V{i}", name=f"V{i}")
              for i in range(16)]
        QT = [persist.tile([128, QB], FP, tag=f"QT{i}", name=f"QT{i}")
              for i in range(4)]
        QN = [persist.tile([128, D], FP, tag=f"QN{i}", name=f"QN{i}")
              for i in range(4)]
        CT = [persist.tile([128, QB], FP, tag=f"CT{i}", name=f"CT{i}")
              for i in range(4)]
        if causal:
            MB = [persist.tile([128, 512], BF, tag=f"MB{i}", name=f"MB{i}")
                  for i in range(4)]
        else:
            MB = [persist.tile([128, S], BF, tag=f"MB{i}", name=f"MB{i}")
                  for i in range(4)]
        wo_s = [wo_pool.tile([128, D], FP, tag=f"wo{i}", name=f"wos{i}")
                for i in range(4)]

        wo_r = d["wo"][:].rearrange("(t p) d -> t p d", p=128)
        for i in range(4):
            nc.sync.dma_start(out=wo_s[i], in_=wo_r[i])
            if causal:
                nc.sync.dma_start(out=MB[i], in_=d["mk"][i])
            else:
                nc.sync.dma_start(out=MB[i],
                                  in_=d["mk"][i * 128:(i + 1) * 128, :])
            nc.sync.dma_start(out=QN[i],
                              in_=d["qn"][i * 128:(i + 1) * 128, :])

        # ---- phase A: projections (scoped pools) ----
        with (
            tc.tile_pool(name="wqkv", bufs=1) as wqkv,
            tc.tile_pool(name="instream", bufs=4) as instream,
            tc.tile_pool(name="pA", bufs=6, space="PSUM") as pA,
        ):
            wq_s = [wqkv.tile([128, D], FP, tag=f"wq{i}", name=f"wqs{i}")
                    for i in range(4)]
            wk_s = [wqkv.tile([128, D], FP, tag=f"wk{i}", name=f"wks{i}")
                    for i in range(4)]
            wv_s = [wqkv.tile([128, D], FP, tag=f"wv{i}", name=f"wvs{i}")
                    for i in range(4)]
            wq_r = d["wq"][:].rearrange("(t p) d -> t p d", p=128)
            wk_r = d["wk"][:].rearrange("(t p) d -> t p d", p=128)
            wv_r = d["wv"][:].rearrange("(t p) d -> t p d", p=128)
            for i in range(4):
                nc.sync.dma_start(out=wq_s[i], in_=wq_r[i])
                nc.sync.dma_start(out=wk_s[i], in_=wk_r[i])
                nc.sync.dma_start(out=wv_s[i], in_=wv_r[i])

            # Q^T projection (bias fused into copy)
            qtin = [instream.tile([128, QB], FP, tag="qtin", name="qtin")
                    for _ in range(4)]
            for din in range(4):
                nc.sync.dma_start(
                    out=qtin[din], in_=d["qT"][din * 128:(din + 1) * 128, :])
            ps_q = [pA.tile([128, QB], FP, tag="pa", name="psq")
                    for _ in range(4)]
            for dout in range(4):
                for din in range(4):
                    nc.tensor.matmul(
                        ps_q[dout], wq_s[din][:, dout * 128:(dout + 1) * 128],
                        qtin[din], start=(din == 0), stop=(din == 3))
                nc.vector.tensor_scalar(
                    out=QT[dout], in0=ps_q[dout],
                    scalar1=bqc[:, dout:dout + 1], scalar2=None,
                    op0=mybir.AluOpType.add)

            # K^T projection, sk in chunks of 512 (bias fused into copy)
            for skc in range(4):
                ktin = [instream.tile([128, 512], FP, tag="ktin",
                                      name="ktin", bufs=6) for _ in range(4)]
                for din in range(4):
                    nc.sync.dma_start(
                        out=ktin[din],
                        in_=d["kT"][din * 128:(din + 1) * 128,
                                    skc * 512:(skc + 1) * 512])
                ps_k = [pA.tile([128, 512], FP, tag="pa", name="psk")
                        for _ in range(4)]
                for dout in range(4):
                    for din in range(4):
                        nc.tensor.matmul(
                            ps_k[dout],
                            wk_s[din][:, dout * 128:(dout + 1) * 128],
                            ktin[din], start=(din == 0), stop=(din == 3))
                    nc.vector.tensor_scalar(
                        out=KT[dout][:, skc * 512:(skc + 1) * 512],
                        in0=ps_k[dout], scalar1=bkc[:, dout:dout + 1],
                        scalar2=None, op0=mybir.AluOpType.add)

            # V projection (bias via broadcast add in the copy)
            for skc in range(4):
                vtin = [instream.tile([128, 512], FP, tag="vtin",
                                      name="vtin", bufs=6) for _ in range(4)]
                for din in range(4):
                    nc.sync.dma_start(
                        out=vtin[din],
                        in_=d["vT"][din * 128:(din + 1) * 128,
                                    skc * 512:(skc + 1) * 512])
                for st in range(4):
                    sk = skc * 4 + st
                    ps_v = pA.tile([128, D], FP, tag="pa", name="psv")
                    for din in range(4):
                        nc.tensor.matmul(
                            ps_v, vtin[din][:, st * 128:(st + 1) * 128],
                            wv_s[din], start=(din == 0), stop=(din == 3))
                    nc.vector.tensor_add(Vt[sk], ps_v, bv_b)

        # ---- hot phase ----
        with (
            tc.tile_pool(name="Pp", bufs=3) as Pp,
            tc.tile_pool(name="PTp", bufs=2) as PTp,
            tc.tile_pool(name="sp", bufs=8) as sp,
            tc.tile_pool(name="Osb", bufs=2) as Osb,
            tc.tile_pool(name="Lp", bufs=2, space="PSUM") as Lp,
            tc.tile_pool(name="Tp", bufs=2, space="PSUM") as Tp,
            tc.tile_pool(name="Cp", bufs=2, space="PSUM") as Cp,
        ):
            for h in range(8):
                ht = h // 2
                hp = (h % 2) * 64
                for qt in range(4):
                    nch = (qt + 1) if causal else 4   # live 512-chunks
                    live = nch * 512
                    P_t = Pp.tile([128, S], FP, tag="p", name="P_t")
                    s_parts = []
                    # logits psum in halves of up to 1024 cols
                    for h0 in range(0, nch, 2):
                        hw = min(2, nch - h0) * 512
                        L_t = Lp.tile([128, 1024], FP, tag="L", name="L_t")
                        for sc in range(h0, min(h0 + 2, nch)):
                            sl = slice((sc - h0) * 512, (sc - h0 + 1) * 512)
                            nc.tensor.matmul(
                                L_t[:, sl],
                                QT[ht][hp:hp + 64, qt * 128:(qt + 1) * 128],
                                KT[ht][hp:hp + 64, sc * 512:(sc + 1) * 512],
                                start=True,
                                stop=(causal and sc != qt))
                            if causal:
                                if sc == qt:
                                    nc.tensor.matmul(
                                        L_t[:, sl], i80k, MB[qt],
                                        start=False, stop=True)
                            else:
                                nc.tensor.matmul(
                                    L_t[:, sl], i80k,
                                    MB[qt][:, sc * 512:(sc + 1) * 512],
                                    start=False, stop=True)
                        sh = sp.tile([128, 1], FP, tag="sh", name="sh")
                        nc.scalar.activation(
                            out=P_t[:, h0 * 512:h0 * 512 + hw],
                            in_=L_t[:, :hw],
                            func=mybir.ActivationFunctionType.Exp,
                            scale=0.125, accum_out=sh)
                        s_parts.append(sh)
                    rs = sp.tile([128, 1], FP, tag="rs", name="rs")
                    if len(s_parts) == 1:
                        nc.vector.reciprocal(rs, s_parts[0])
                    else:
                        nc.vector.tensor_add(rs, s_parts[0], s_parts[1])
                        nc.vector.reciprocal(rs, rs)
                    nc.vector.tensor_scalar_mul(
                        P_t[:, :live], P_t[:, :live], rs)
                    nc.sync.dma_start(
                        out=d["attn_o"][h, qt * 128:(qt + 1) * 128, :live],
                        in_=P_t[:, :live])
                    if causal and live < S:
                        nc.sync.dma_start(
                            out=d["attn_o"][h, qt * 128:(qt + 1) * 128,
                                            live:],
                            in_=zerot[:, :S - live])
                    # transpose live 128-blocks -> PT
                    PT_t = PTp.tile([128, S], FP, tag="pt", name="PT_t")
                    nkt = nch * 4
                    for jg in range((nkt + 3) // 4):
                        T_t = Tp.tile([128, 512], FP, tag="tp", name="T_t")
                        n_in_g = min(4, nkt - jg * 4)
                        for jj in range(n_in_g):
                            j = jg * 4 + jj
                            nc.tensor.transpose(
                                T_t[:, jj * 128:(jj + 1) * 128],
                                P_t[:, j * 128:(j + 1) * 128], ident)
                        dst = PT_t[:, jg * 512:jg * 512 + n_in_g * 128]
                        if jg % 2 == 0:
                            nc.vector.tensor_copy(dst, T_t[:, :n_in_g * 128])
                        else:
                            nc.scalar.copy(dst, T_t[:, :n_in_g * 128])
                    # PV: ctx^T[64, 128q] over live k-tiles
                    ct = Cp.tile([64, 128], FP, tag="ctx", name="ct")
                    for j in range(nkt):
                        nc.tensor.matmul(
                            ct, Vt[j][:, h * 64:(h + 1) * 64],
                            PT_t[:, j * 128:(j + 1) * 128],
                            start=(j == 0), stop=(j == nkt - 1))
                    nc.vector.tensor_copy(
                        CT[ht][hp:hp + 64, qt * 128:(qt + 1) * 128], ct)

            # ---- phase C: output projection + residual + LayerNorm ----
            for qt in range(4):
                O_ps = Tp.tile([128, 512], FP, tag="tp", name="O_ps")
                for din in range(4):
                    nc.tensor.matmul(
                        O_ps, CT[din][:, qt * 128:(qt + 1) * 128],
                        wo_s[din], start=(din == 0), stop=(din == 3))
                O_sb = Osb.tile([128, D], FP, tag="osb", name="O_sb")
                nc.vector.tensor_add(O_sb, O_ps, QN[qt])
                stats = sp.tile([128, 6], FP, tag="st", name="stats")
                mv = sp.tile([128, 2], FP, tag="mv", name="mv")
                nc.vector.bn_stats(stats, O_sb)
                nc.vector.bn_aggr(mv, stats)
                rstd = sp.tile([128, 1], FP, tag="rstd", name="rstd")
                nc.scalar.activation(
                    out=rstd, in_=mv[:, 1:2],
                    func=mybir.ActivationFunctionType.Sqrt, bias=eps_t)
                nc.vector.reciprocal(rstd, rstd)
                nc.vector.tensor_scalar(
                    out=O_sb, in0=O_sb, scalar1=mv[:, 0:1], scalar2=rstd,
                    op0=mybir.AluOpType.subtract, op1=mybir.AluOpType.mult)
                nc.vector.tensor_mul(O_sb, O_sb, ga_b)
                nc.vector.tensor_add(O_sb, O_sb, be_b)
                nc.sync.dma_start(
                    out=d["out_o"][qt * 128:(qt + 1) * 128, :], in_=O_sb)


_NC_CACHE = {}


def _get_nc(causal):
    if causal not in _NC_CACHE:
        _NC_CACHE[causal] = _build_nc(causal)
    return _NC_CACHE[causal]


def _qtiles(j):
    return sorted([j, 7 - j, 8 + j, 15 - j])


def kernel(query, key, value, mask, wq, bq, wk, bk, wv, bv, wo, bo,
           gamma, beta):
    query = np.ascontiguousarray(np.asarray(query, np.float32))
    key = np.ascontiguousarray(np.asarray(key, np.float32))
    value = np.ascontiguousarray(np.asarray(value, np.float32))
    mask = np.ascontiguousarray(np.asarray(mask, np.float32))
    wq = np.ascontiguousarray(np.asarray(wq, np.float32))
    wk = np.ascontiguousarray(np.asarray(wk, np.float32))
    wv = np.ascontiguousarray(np.asarray(wv, np.float32))
    wo = np.ascontiguousarray(np.asarray(wo, np.float32))
    bq = np.asarray(bq, np.float32)
    bk = np.asarray(bk, np.float32)
    bv = np.asarray(bv, np.float32)
    bo = np.asarray(bo, np.float32)
    gamma = np.asarray(gamma, np.float32)
    beta = np.asarray(beta, np.float32)

    causal_ref = np.triu(np.ones((S, S), np.float32), k=1)
    causal = all(np.array_equal(mask[b, 0], causal_ref) for b in range(B))

    nc = _get_nc(causal)
    in_maps = []
    for c in range(NCORES):
        b = c // 4
        j = c % 4
        tiles = _qtiles(j)
        qrows = np.concatenate(
            [np.arange(g * 128, (g + 1) * 128) for g in tiles])
        qs = query[b][qrows]
        if causal:
            mk = np.stack([
                mask[b, 0, tiles[t] * 128:(tiles[t] + 1) * 128,
                     t * 512:(t + 1) * 512]
                for t in range(NQT)]).astype(ml_dtypes.bfloat16)
        else:
            mk = mask[b, 0][qrows].astype(ml_dtypes.bfloat16)
        in_maps.append({
            "qT": np.ascontiguousarray(qs.T),
            "qn": np.ascontiguousarray(qs + bo[None, :]),
            "kT": np.ascontiguousarray(key[b].T),
            "vT": np.ascontiguousarray(value[b].T),
            "mk": np.ascontiguousarray(mk),
            "wq": wq, "wk": wk, "wv": wv, "wo": wo,
            "bq": bq, "bk": bk, "bv": bv,
            "ga": gamma, "be": beta,
        })

    r = run_bass_kernel_spmd(nc, in_maps, core_ids=list(range(NCORES)),
                             trace=TRACE)
    if TRACE:
        kernel.last_results = r

    out = np.empty((B, S, D), np.float32)
    attn = np.empty((B, H, S, S), np.float32)
    for c in range(NCORES):
        b = c // 4
        tiles = _qtiles(c % 4)
        for t, g in enumerate(tiles):
            out[b, g * 128:(g + 1) * 128, :] = \
                r.results[c]["out_o"][t * 128:(t + 1) * 128]
            attn[b, :, g * 128:(g + 1) * 128, :] = \
                r.results[c]["attn_o"][:, t * 128:(t + 1) * 128, :]
    return out, attn


# revision 18
# speedup vs baseline: 181408.4048x; 95469.7349x over previous
"""Multi-head attention (B=2,S=2048,D=512,H=8,depth=64) + causal-mask softmax
+ output projection + residual + LayerNorm, returning (out, attn).

Sharding: sequence-parallel over query rows, causal-load-balanced. 8 cores;
core c handles batch b = c // 4 and the four global 128-row q-tiles
{j, 7-j, 8+j, 15-j} (j = c % 4). With a causal mask, local q-tile t then
has exactly t+1 live 512-wide k-chunks on EVERY core, so the SPMD program
is uniform while skipping the fully-masked upper-triangular blocks.
A dense variant (all chunks live, additive mask everywhere) is built when
the host detects the mask is not exactly causal.

Per-core device pipeline:
  - host uploads qT [D,512](transposed q rows), qn (= q rows + bo), kT, vT,
    mask chunks (bf16), weights, biases, ln params.
  - Q^T = wq^T @ qT, K^T = wk^T @ kT (bias fused into the PSUM->SBUF copy
    as a per-partition tensor_scalar add), V = vT_tiles^T @ wv (bias via
    broadcast tensor_add in the copy).
  - logits psum = Q_h^T.T @ K_h^T (fp32) + bf16 identity-matmul of
    (-79872*mask) for masked chunks; exp on ScalarE (scale=1/8) with
    accum_out row-sums; reciprocal; normalize in-place (tensor_scalar).
  - attn output DMA (plus a zero-tile DMA for skipped chunks).
  - P^T via PE transposes; PV matmul -> normalized ctx^T [64, q] per head.
  - O = ctx^T.T @ wo + residual(qn) then LayerNorm via bn_stats -> out DMA.
"""

import numpy as np
import ml_dtypes

import concourse.bacc as bacc
import concourse.tile as tile
from concourse import mybir
from concourse.masks import make_identity
from concourse.bass_utils import run_bass_kernel_spmd

B, S, D, H, DEPTH = 2, 2048, 512, 8, 64
QB = 512               # query rows per core
NQT = QB // 128        # local q-tiles per core
NCORES = 8
FP = mybir.dt.float32
BF = mybir.dt.bfloat16
MASK_SCALE = -80000.0  # ~ -10000 * 8 (exp applies scale=1/8); bf16-rounded
EPS = 1e-6

TRACE = False          # test.py sets this for profiled runs


def _build_nc(causal: bool):
    nc = bacc.Bacc(None, target_bir_lowering=False)

    # ---- I/O ----
    d = {}
    d["qT"] = nc.dram_tensor("qT", [D, QB], FP, kind="ExternalInput")
    d["qn"] = nc.dram_tensor("qn", [QB, D], FP, kind="ExternalInput")
    d["kT"] = nc.dram_tensor("kT", [D, S], FP, kind="ExternalInput")
    d["vT"] = nc.dram_tensor("vT", [D, S], FP, kind="ExternalInput")
    if causal:
        # per local q-tile: only the diagonal 512-chunk of the mask
        d["mk"] = nc.dram_tensor("mk", [NQT, 128, 512], BF,
                                 kind="ExternalInput")
    else:
        d["mk"] = nc.dram_tensor("mk", [QB, S], BF, kind="ExternalInput")
    for w in ("wq", "wk", "wv", "wo"):
        d[w] = nc.dram_tensor(w, [D, D], FP, kind="ExternalInput")
    for bnm in ("bq", "bk", "bv", "ga", "be"):
        d[bnm] = nc.dram_tensor(bnm, [D], FP, kind="ExternalInput")
    d["attn_o"] = nc.dram_tensor("attn_o", [H, QB, S], FP,
                                 kind="ExternalOutput")
    d["out_o"] = nc.dram_tensor("out_o", [QB, D], FP, kind="ExternalOutput")

    with tile.TileContext(nc) as tc:
        _emit(nc, tc, d, causal)
    nc.compile()
    return nc


def _emit(nc, tc, d, causal):
    from contextlib import ExitStack

    es = ExitStack()
    with es:
        consts = es.enter_context(tc.tile_pool(name="consts", bufs=1))
        persist = es.enter_context(tc.tile_pool(name="persist", bufs=1))
        wo_pool = es.enter_context(tc.tile_pool(name="wo", bufs=1))

        # ---- constants ----
        ident = consts.tile([128, 128], FP)
        make_identity(nc, ident)
        i80k = consts.tile([128, 128], BF)
        make_identity(nc, i80k)
        nc.vector.tensor_scalar_mul(i80k, i80k, MASK_SCALE)
        eps_t = consts.tile([128, 1], FP)
        nc.vector.memset(eps_t, EPS)
        # bias columns [128, 4]: bq/bk rearranged so tile t's per-partition
        # bias column is bqc[:, t]
        bqc = consts.tile([128, 4], FP)
        bkc = consts.tile([128, 4], FP)
        nc.sync.dma_start(out=bqc, in_=d["bq"][:].rearrange("(t p) -> p t",
                                                            p=128))
        nc.sync.dma_start(out=bkc, in_=d["bk"][:].rearrange("(t p) -> p t",
                                                            p=128))
        bv_b = consts.tile([128, D], FP)
        nc.sync.dma_start(out=bv_b, in_=d["bv"][None, :].to_broadcast(
            [128, D]))
        ga_b = consts.tile([128, D], FP)
        be_b = consts.tile([128, D], FP)
        nc.sync.dma_start(out=ga_b, in_=d["ga"][None, :].to_broadcast(
            [128, D]))
        nc.sync.dma_start(out=be_b, in_=d["be"][None, :].to_broadcast(
            [128, D]))
        if causal:
            zerot = consts.tile([128, 1536], FP)
            nc.vector.memset(zerot, 0.0)

        # ---- persistent SBUF ----
        KT = [persist.tile([128, S], FP, tag=f"KT{i}", name=f"KT{i}")
              for i in range(4)]
        Vt = [persist.tile([128, D], FP, tag=f"V{i}", name=f"V{i}")
              for i in range(16)]
        QT = [persist.tile([128, QB], FP, tag=f"QT{i}", name=f"QT{i}")
              for i in range(4)]
        QN = [persist.tile([128, D], FP, tag=f"QN{i}", name=f"QN{i}")
              for i in range(4)]
        CT = [persist.tile([128, QB], FP, tag=f"CT{i}", name=f"CT{i}")
              for i in range(4)]
        if causal:
            MB = [persist.tile([128, 512], BF, tag=f"MB{i}", name=f"MB{i}")
                  for i in range(4)]
        else:
            MB = [persist.tile([128, S], BF, tag=f"MB{i}", name=f"MB{i}")
                  for i in range(4)]
        wo_s = [wo_pool.tile([128, D], FP, tag=f"wo{i}", name=f"wos{i}")
                for i in range(4)]

        wo_r = d["wo"][:].rearrange("(t p) d -> t p d", p=128)
        for i in range(4):
            nc.sync.dma_start(out=wo_s[i], in_=wo_r[i])
            if causal:
                nc.sync.dma_start(out=MB[i], in_=d["mk"][i])
            else:
                nc.sync.dma_start(out=MB[i],
                                  in_=d["mk"][i * 128:(i + 1) * 128, :])
            nc.sync.dma_start(out=QN[i],
                              in_=d["qn"][i * 128:(i + 1) * 128, :])

        # ---- phase A: projections (scoped pools) ----
        with (
            tc.tile_pool(name="wqkv", bufs=1) as wqkv,
            tc.tile_pool(name="instream", bufs=4) as instream,
            tc.tile_pool(name="pA", bufs=6, space="PSUM") as pA,
        ):
            wq_s = [wqkv.tile([128, D], FP, tag=f"wq{i}", name=f"wqs{i}")
                    for i in range(4)]
            wk_s = [wqkv.tile([128, D], FP, tag=f"wk{i}", name=f"wks{i}")
                    for i in range(4)]
            wv_s = [wqkv.tile([128, D], FP, tag=f"wv{i}", name=f"wvs{i}")
                    for i in range(4)]
            wq_r = d["wq"][:].rearrange("(t p) d -> t p d", p=128)
            wk_r = d["wk"][:].rearrange("(t p) d -> t p d", p=128)
            wv_r = d["wv"][:].rearrange("(t p) d -> t p d", p=128)
            for i in range(4):
                nc.sync.dma_start(out=wq_s[i], in_=wq_r[i])
                nc.sync.dma_start(out=wk_s[i], in_=wk_r[i])
                nc.sync.dma_start(out=wv_s[i], in_=wv_r[i])

            # Q^T projection (bias fused into copy)
            qtin = [instream.tile([128, QB], FP, tag="qtin", name="qtin")
                    for _ in range(4)]
            for din in range(4):
                nc.sync.dma_start(
                    out=qtin[din], in_=d["qT"][din * 128:(din + 1) * 128, :])
            ps_q = [pA.tile([128, QB], FP, tag="pa", name="psq")
                    for _ in range(4)]
            for dout in range(4):
                for din in range(4):
                    nc.tensor.matmul(
                        ps_q[dout], wq_s[din][:, dout * 128:(dout + 1) * 128],
                        qtin[din], start=(din == 0), stop=(din == 3))
                nc.vector.tensor_scalar(
                    out=QT[dout], in0=ps_q[dout],
                    scalar1=bqc[:, dout:dout + 1], scalar2=None,
                    op0=mybir.AluOpType.add)

            # K^T projection, sk in chunks of 512 (bias fused into copy)
            for skc in range(4):
                ktin = [instream.tile([128, 512], FP, tag="ktin",
                                      name="ktin", bufs=6) for _ in range(4)]
                for din in range(4):
                    nc.sync.dma_start(
                        out=ktin[din],
                        in_=d["kT"][din * 128:(din + 1) * 128,
                                    skc * 512:(skc + 1) * 512])
                ps_k = [pA.tile([128, 512], FP, tag="pa", name="psk")
                        for _ in range(4)]
                for dout in range(4):
                    for din in range(4):
                        nc.tensor.matmul(
                            ps_k[dout],
                            wk_s[din][:, dout * 128:(dout + 1) * 128],
                            ktin[din], start=(din == 0), stop=(din == 3))
                    nc.vector.tensor_scalar(
                        out=KT[dout][:, skc * 512:(skc + 1) * 512],
                        in0=ps_k[dout], scalar1=bkc[:, dout:dout + 1],
                        scalar2=None, op0=mybir.AluOpType.add)

            # V projection (bias via broadcast add in the copy)
            for skc in range(4):
                vtin = [instream.tile([128, 512], FP, tag="vtin",
                                      name="vtin", bufs=6) for _ in range(4)]
                for din in range(4):
                    nc.sync.dma_start(
                        out=vtin[din],
                        in_=d["vT"][din * 128:(din + 1) * 128,
                                    skc * 512:(skc + 1) * 512])
                for st in range(4):
                    sk = skc * 4 + st
                    ps_v = pA.tile([128, D], FP, tag="pa", name="psv")
                    for din in range(4):
                        nc.tensor.matmul(
                            ps_v, vtin[din][:, st * 128:(st + 1) * 128],
                            wv_s[din], start=(din == 0), stop=(din == 3))
                    nc.vector.tensor_add(Vt[sk], ps_v, bv_b)

        # ---- hot phase ----
        with (
            tc.tile_pool(name="Pp", bufs=3) as Pp,
            tc.tile_pool(name="PTp", bufs=2) as PTp,
            tc.tile_pool(name="sp", bufs=8) as sp,
            tc.tile_pool(name="Osb", bufs=2) as Osb,
            tc.tile_pool(name="Lp", bufs=2, space="PSUM") as Lp,
            tc.tile_pool(name="Tp", bufs=2, space="PSUM") as Tp,
            tc.tile_pool(name="Cp", bufs=2, space="PSUM") as Cp,
        ):
            for qt in range(4):
                for h in range(8):
                    ht = h // 2
                    hp = (h % 2) * 64
                    nch = (qt + 1) if causal else 4   # live 512-chunks
                    live = nch * 512
                    P_t = Pp.tile([128, S], FP, tag="p", name="P_t")
                    s_parts = []
                    # logits psum in halves of up to 1024 cols
                    for h0 in range(0, nch, 2):
                        hw = min(2, nch - h0) * 512
                        L_t = Lp.tile([128, 1024], FP, tag="L", name="L_t")
                        for sc in range(h0, min(h0 + 2, nch)):
                            sl = slice((sc - h0) * 512, (sc - h0 + 1) * 512)
                            nc.tensor.matmul(
                                L_t[:, sl],
                                QT[ht][hp:hp + 64, qt * 128:(qt + 1) * 128],
                                KT[ht][hp:hp + 64, sc * 512:(sc + 1) * 512],
                                start=True,
                                stop=(causal and sc != qt))
                            if causal:
                                if sc == qt:
                                    nc.tensor.matmul(
                                        L_t[:, sl], i80k, MB[qt],
                                        start=False, stop=True)
                            else:
                                nc.tensor.matmul(
                                    L_t[:, sl], i80k,
                                    MB[qt][:, sc * 512:(sc + 1) * 512],
                                    start=False, stop=True)
                        sh = sp.tile([128, 1], FP, tag="sh", name="sh")
                        nc.scalar.activation(
                            out=P_t[:, h0 * 512:h0 * 512 + hw],
                            in_=L_t[:, :hw],
                            func=mybir.ActivationFunctionType.Exp,
                            scale=0.125, accum_out=sh)
                        s_parts.append(sh)
                    rs = sp.tile([128, 1], FP, tag="rs", name="rs")
                    if len(s_parts) == 1:
                        nc.vector.reciprocal(rs, s_parts[0])
                    else:
                        nc.vector.tensor_add(rs, s_parts[0], s_parts[1])
                        nc.vector.reciprocal(rs, rs)
                    nc.vector.tensor_scalar_mul(
                        P_t[:, :live], P_t[:, :live], rs)
                    nc.sync.dma_start(
                        out=d["attn_o"][h, qt * 128:(qt + 1) * 128, :live],
                        in_=P_t[:, :live])
                    if causal and live < S:
                        nc.sync.dma_start(
                            out=d["attn_o"][h, qt * 128:(qt + 1) * 128,
                                            live:],
                            in_=zerot[:, :S - live])
                    # transpose live 128-blocks -> PT
                    PT_t = PTp.tile([128, S], FP, tag="pt", name="PT_t")
                    nkt = nch * 4
                    for jg in range((nkt + 3) // 4):
                        T_t = Tp.tile([128, 512], FP, tag="tp", name="T_t")
                        n_in_g = min(4, nkt - jg * 4)
                        for jj in range(n_in_g):
                            j = jg * 4 + jj
                            nc.tensor.transpose(
                                T_t[:, jj * 128:(jj + 1) * 128],
                                P_t[:, j * 128:(j + 1) * 128], ident)
                        dst = PT_t[:, jg * 512:jg * 512 + n_in_g * 128]
                        if jg % 2 == 0:
                            nc.vector.tensor_copy(dst, T_t[:, :n_in_g * 128])
                        else:
                            nc.scalar.copy(dst, T_t[:, :n_in_g * 128])
                    # PV: ctx^T[64, 128q] over live k-tiles
                    ct = Cp.tile([64, 128], FP, tag="ctx", name="ct")
                    for j in range(nkt):
                        nc.tensor.matmul(
                            ct, Vt[j][:, h * 64:(h + 1) * 64],
                            PT_t[:, j * 128:(j + 1) * 128],
                            start=(j == 0), stop=(j == nkt - 1))
                    nc.vector.tensor_copy(
                        CT[ht][hp:hp + 64, qt * 128:(qt + 1) * 128], ct)

                # ---- output projection + residual + LayerNorm for qt ----
                O_ps = Tp.tile([128, 512], FP, tag="tp", name="O_ps")
                for din in range(4):
                    nc.tensor.matmul(
                        O_ps, CT[din][:, qt * 128:(qt + 1) * 128],
                        wo_s[din], start=(din == 0), stop=(din == 3))
                O_sb = Osb.tile([128, D], FP, tag="osb", name="O_sb")
                nc.vector.tensor_add(O_sb, O_ps, QN[qt])
                stats = sp.tile([128, 6], FP, tag="st", name="stats")
                mv = sp.tile([128, 2], FP, tag="mv", name="mv")
                nc.vector.bn_stats(stats, O_sb)
                nc.vector.bn_aggr(mv, stats)
                rstd = sp.tile([128, 1], FP, tag="rstd", name="rstd")
                nc.scalar.activation(
                    out=rstd, in_=mv[:, 1:2],
                    func=mybir.ActivationFunctionType.Sqrt, bias=eps_t)
                nc.vector.reciprocal(rstd, rstd)
                nc.vector.tensor_scalar(
                    out=O_sb, in0=O_sb, scalar1=mv[:, 0:1], scalar2=rstd,
                    op0=mybir.AluOpType.subtract, op1=mybir.AluOpType.mult)
                nc.vector.tensor_mul(O_sb, O_sb, ga_b)
                nc.vector.tensor_add(O_sb, O_sb, be_b)
                nc.sync.dma_start(
                    out=d["out_o"][qt * 128:(qt + 1) * 128, :], in_=O_sb)


_NC_CACHE = {}


def _get_nc(causal):
    if causal not in _NC_CACHE:
        _NC_CACHE[causal] = _build_nc(causal)
    return _NC_CACHE[causal]


def _qtiles(j):
    return sorted([j, 7 - j, 8 + j, 15 - j])


def kernel(query, key, value, mask, wq, bq, wk, bk, wv, bv, wo, bo,
           gamma, beta):
    query = np.ascontiguousarray(np.asarray(query, np.float32))
    key = np.ascontiguousarray(np.asarray(key, np.float32))
    value = np.ascontiguousarray(np.asarray(value, np.float32))
    mask = np.ascontiguousarray(np.asarray(mask, np.float32))
    wq = np.ascontiguousarray(np.asarray(wq, np.float32))
    wk = np.ascontiguousarray(np.asarray(wk, np.float32))
    wv = np.ascontiguousarray(np.asarray(wv, np.float32))
    wo = np.ascontiguousarray(np.asarray(wo, np.float32))
    bq = np.asarray(bq, np.float32)
    bk = np.asarray(bk, np.float32)
    bv = np.asarray(bv, np.float32)
    bo = np.asarray(bo, np.float32)
    gamma = np.asarray(gamma, np.float32)
    beta = np.asarray(beta, np.float32)

    causal_ref = np.triu(np.ones((S, S), np.float32), k=1)
    causal = all(np.array_equal(mask[b, 0], causal_ref) for b in range(B))

    nc = _get_nc(causal)
    in_maps = []
    for c in range(NCORES):
        b = c // 4
        j = c % 4
        tiles = _qtiles(j)
        qrows = np.concatenate(
            [np.arange(g * 128, (g + 1) * 128) for g in tiles])
        qs = query[b][qrows]
        if causal:
            mk = np.stack([
                mask[b, 0, tiles[t] * 128:(tiles[t] + 1) * 128,
                     t * 512:(t + 1) * 512]
                for t in range(NQT)]).astype(ml_dtypes.bfloat16)
        else:
            mk = mask[b, 0][qrows].astype(ml_dtypes.bfloat16)
        in_maps.append({
            "qT": np.ascontiguousarray(qs.T),
            "qn": np.ascontiguousarray(qs + bo[None, :]),
            "kT": np.ascontiguousarray(key[b].T),
            "vT": np.ascontiguousarray(value[b].T),
            "mk": np.ascontiguousarray(mk),
            "wq": wq, "wk": wk, "wv": wv, "wo": wo,
            "bq": bq, "bk": bk, "bv": bv,
            "ga": gamma, "be": beta,
        })

    r = run_bass_kernel_spmd(nc, in_maps, core_ids=list(range(NCORES)),
                             trace=TRACE)
    if TRACE:
        kernel.last_results = r

    out = np.empty((B, S, D), np.float32)
    attn = np.empty((B, H, S, S), np.float32)
    for c in range(NCORES):
        b = c // 4
        tiles = _qtiles(c % 4)
        for t, g in enumerate(tiles):
            out[b, g * 128:(g + 1) * 128, :] = \
                r.results[c]["out_o"][t * 128:(t + 1) * 128]
            attn[b, :, g * 128:(g + 1) * 128, :] = \
                r.results[c]["attn_o"][:, t * 128:(t + 1) * 128, :]
    return out, attn


# revision 19
# speedup vs baseline: 192084.8668x; 1.0589x over previous
"""Multi-head attention (B=2,S=2048,D=512,H=8,depth=64) + causal-mask softmax
+ output projection + residual + LayerNorm, returning (out, attn).

Sharding: sequence-parallel over query rows, causal-load-balanced. 8 cores;
core c handles batch b = c // 4 and the four global 128-row q-tiles
{j, 7-j, 8+j, 15-j} (j = c % 4). With a causal mask, local q-tile t then
has exactly t+1 live 512-wide k-chunks on EVERY core, so the SPMD program
is uniform while skipping the fully-masked upper-triangular blocks.
A dense variant (all chunks live, additive mask everywhere) is built when
the host detects the mask is not exactly causal.

Per-core device pipeline:
  - host uploads qT [D,512](transposed q rows), qn (= q rows + bo), kT, vT,
    mask chunks (bf16), weights, biases, ln params.
  - Q^T = wq^T @ qT, K^T = wk^T @ kT (bias fused into the PSUM->SBUF copy
    as a per-partition tensor_scalar add), V = vT_tiles^T @ wv (bias via
    broadcast tensor_add in the copy).
  - logits psum = Q_h^T.T @ K_h^T (fp32) + bf16 identity-matmul of
    (-79872*mask) for masked chunks; exp on ScalarE (scale=1/8) with
    accum_out row-sums; reciprocal; normalize in-place (tensor_scalar).
  - attn output DMA (plus a zero-tile DMA for skipped chunks).
  - P^T via PE transposes; PV matmul -> normalized ctx^T [64, q] per head.
  - O = ctx^T.T @ wo + residual(qn) then LayerNorm via bn_stats -> out DMA.
"""

import numpy as np
import ml_dtypes

import concourse.bacc as bacc
import concourse.tile as tile
from concourse import mybir
from concourse.masks import make_identity
from concourse.bass_utils import run_bass_kernel_spmd

B, S, D, H, DEPTH = 2, 2048, 512, 8, 64
QB = 512               # query rows per core
NQT = QB // 128        # local q-tiles per core
NCORES = 8
FP = mybir.dt.float32
BF = mybir.dt.bfloat16
MASK_SCALE = -80000.0  # ~ -10000 * 8 (exp applies scale=1/8); bf16-rounded
EPS = 1e-6

TRACE = False          # test.py sets this for profiled runs


def _build_nc(causal: bool):
    nc = bacc.Bacc(None, target_bir_lowering=False)

    # ---- I/O ----
    d = {}
    d["qT"] = nc.dram_tensor("qT", [D, QB], FP, kind="ExternalInput")
    d["qn"] = nc.dram_tensor("qn", [QB, D], FP, kind="ExternalInput")
    d["kT"] = nc.dram_tensor("kT", [D, S], FP, kind="ExternalInput")
    d["vT"] = nc.dram_tensor("vT", [D, S], FP, kind="ExternalInput")
    if causal:
        # per local q-tile: only the diagonal 512-chunk of the mask
        d["mk"] = nc.dram_tensor("mk", [NQT, 128, 512], BF,
                                 kind="ExternalInput")
    else:
        d["mk"] = nc.dram_tensor("mk", [QB, S], BF, kind="ExternalInput")
    for w in ("wq", "wk", "wv", "wo"):
        d[w] = nc.dram_tensor(w, [D, D], FP, kind="ExternalInput")
    for bnm in ("bq", "bk", "bv", "ga", "be"):
        d[bnm] = nc.dram_tensor(bnm, [D], FP, kind="ExternalInput")
    d["attn_o"] = nc.dram_tensor("attn_o", [H, QB, S], FP,
                                 kind="ExternalOutput")
    d["out_o"] = nc.dram_tensor("out_o", [QB, D], FP, kind="ExternalOutput")

    with tile.TileContext(nc) as tc:
        _emit(nc, tc, d, causal)
    nc.compile()
    return nc


def _emit(nc, tc, d, causal):
    from contextlib import ExitStack

    es = ExitStack()
    with es:
        consts = es.enter_context(tc.tile_pool(name="consts", bufs=1))
        persist = es.enter_context(tc.tile_pool(name="persist", bufs=1))
        wo_pool = es.enter_context(tc.tile_pool(name="wo", bufs=1))

        # ---- constants ----
        ident = consts.tile([128, 128], FP)
        make_identity(nc, ident)
        i80k = consts.tile([128, 128], BF)
        make_identity(nc, i80k)
        nc.vector.tensor_scalar_mul(i80k, i80k, MASK_SCALE)
        eps_t = consts.tile([128, 1], FP)
        nc.vector.memset(eps_t, EPS)
        # bias columns [128, 4]: bq/bk rearranged so tile t's per-partition
        # bias column is bqc[:, t]
        bqc = consts.tile([128, 4], FP)
        bkc = consts.tile([128, 4], FP)
        nc.sync.dma_start(out=bqc, in_=d["bq"][:].rearrange("(t p) -> p t",
                                                            p=128))
        nc.sync.dma_start(out=bkc, in_=d["bk"][:].rearrange("(t p) -> p t",
                                                            p=128))
        bv_b = consts.tile([128, D], FP)
        ga_b = consts.tile([128, D], FP)
        be_b = consts.tile([128, D], FP)
        if causal:
            zerot = consts.tile([128, 1536], FP)
            nc.vector.memset(zerot, 0.0)

        # ---- persistent SBUF ----
        KT = [persist.tile([128, S], FP, tag=f"KT{i}", name=f"KT{i}")
              for i in range(4)]
        Vt = [persist.tile([128, D], FP, tag=f"V{i}", name=f"V{i}")
              for i in range(16)]
        QT = [persist.tile([128, QB], FP, tag=f"QT{i}", name=f"QT{i}")
              for i in range(4)]
        QN = [persist.tile([128, D], FP, tag=f"QN{i}", name=f"QN{i}")
              for i in range(4)]
        CT = [persist.tile([128, QB], FP, tag=f"CT{i}", name=f"CT{i}")
              for i in range(4)]
        if causal:
            MB = [persist.tile([128, 512], BF, tag=f"MB{i}", name=f"MB{i}")
                  for i in range(4)]
        else:
            MB = [persist.tile([128, S], BF, tag=f"MB{i}", name=f"MB{i}")
                  for i in range(4)]
        wo_s = [wo_pool.tile([128, D], FP, tag=f"wo{i}", name=f"wos{i}")
                for i in range(4)]

        # ---- phase A: projections (scoped pools) ----
        with (
            tc.tile_pool(name="wqkv", bufs=1) as wqkv,
            tc.tile_pool(name="instream", bufs=4) as instream,
            tc.tile_pool(name="pA", bufs=6, space="PSUM") as pA,
        ):
            wq_s = [wqkv.tile([128, D], FP, tag=f"wq{i}", name=f"wqs{i}")
                    for i in range(4)]
            wk_s = [wqkv.tile([128, D], FP, tag=f"wk{i}", name=f"wks{i}")
                    for i in range(4)]
            wv_s = [wqkv.tile([128, D], FP, tag=f"wv{i}", name=f"wvs{i}")
                    for i in range(4)]
            wq_r = d["wq"][:].rearrange("(t p) d -> t p d", p=128)
            wk_r = d["wk"][:].rearrange("(t p) d -> t p d", p=128)
            wv_r = d["wv"][:].rearrange("(t p) d -> t p d", p=128)

            # Q^T projection (bias fused into copy); its inputs are DMA'd
            # first so the PE unblocks as early as possible
            qtin = [instream.tile([128, QB], FP, tag="qtin", name="qtin")
                    for _ in range(4)]
            for din in range(4):
                nc.sync.dma_start(out=wq_s[din], in_=wq_r[din])
                nc.sync.dma_start(
                    out=qtin[din], in_=d["qT"][din * 128:(din + 1) * 128, :])
            ps_q = [pA.tile([128, QB], FP, tag="pa", name="psq")
                    for _ in range(4)]
            for dout in range(4):
                for din in range(4):
                    nc.tensor.matmul(
                        ps_q[dout], wq_s[din][:, dout * 128:(dout + 1) * 128],
                        qtin[din], start=(din == 0), stop=(din == 3))
                nc.vector.tensor_scalar(
                    out=QT[dout], in0=ps_q[dout],
                    scalar1=bqc[:, dout:dout + 1], scalar2=None,
                    op0=mybir.AluOpType.add)

            # K^T projection, sk in chunks of 512 (bias fused into copy)
            for i in range(4):
                nc.sync.dma_start(out=wk_s[i], in_=wk_r[i])
            for skc in range(4):
                ktin = [instream.tile([128, 512], FP, tag="ktin",
                                      name="ktin", bufs=6) for _ in range(4)]
                for din in range(4):
                    nc.sync.dma_start(
                        out=ktin[din],
                        in_=d["kT"][din * 128:(din + 1) * 128,
                                    skc * 512:(skc + 1) * 512])
                ps_k = [pA.tile([128, 512], FP, tag="pa", name="psk")
                        for _ in range(4)]
                for dout in range(4):
                    for din in range(4):
                        nc.tensor.matmul(
                            ps_k[dout],
                            wk_s[din][:, dout * 128:(dout + 1) * 128],
                            ktin[din], start=(din == 0), stop=(din == 3))
                    nc.vector.tensor_scalar(
                        out=KT[dout][:, skc * 512:(skc + 1) * 512],
                        in0=ps_k[dout], scalar1=bkc[:, dout:dout + 1],
                        scalar2=None, op0=mybir.AluOpType.add)

            for i in range(4):
                if causal:
                    nc.sync.dma_start(out=MB[i], in_=d["mk"][i])
                else:
                    nc.sync.dma_start(out=MB[i],
                                      in_=d["mk"][i * 128:(i + 1) * 128, :])
                nc.sync.dma_start(out=QN[i],
                                  in_=d["qn"][i * 128:(i + 1) * 128, :])
            nc.sync.dma_start(out=bv_b, in_=d["bv"][None, :].to_broadcast(
                [128, D]))
            for i in range(4):
                nc.sync.dma_start(out=wv_s[i], in_=wv_r[i])

            # V projection (bias via broadcast add in the copy)
            for skc in range(4):
                vtin = [instream.tile([128, 512], FP, tag="vtin",
                                      name="vtin", bufs=6) for _ in range(4)]
                for din in range(4):
                    nc.sync.dma_start(
                        out=vtin[din],
                        in_=d["vT"][din * 128:(din + 1) * 128,
                                    skc * 512:(skc + 1) * 512])
                for st in range(4):
                    sk = skc * 4 + st
                    ps_v = pA.tile([128, D], FP, tag="pa", name="psv")
                    for din in range(4):
                        nc.tensor.matmul(
                            ps_v, vtin[din][:, st * 128:(st + 1) * 128],
                            wv_s[din], start=(din == 0), stop=(din == 3))
                    nc.vector.tensor_add(Vt[sk], ps_v, bv_b)

        wo_r2 = d["wo"][:].rearrange("(t p) d -> t p d", p=128)
        for i in range(4):
            nc.sync.dma_start(out=wo_s[i], in_=wo_r2[i])
        nc.sync.dma_start(out=ga_b, in_=d["ga"][None, :].to_broadcast(
            [128, D]))
        nc.sync.dma_start(out=be_b, in_=d["be"][None, :].to_broadcast(
            [128, D]))

        # ---- hot phase ----
        with (
            tc.tile_pool(name="Pp", bufs=3) as Pp,
            tc.tile_pool(name="PTp", bufs=2) as PTp,
            tc.tile_pool(name="sp", bufs=8) as sp,
            tc.tile_pool(name="Osb", bufs=2) as Osb,
            tc.tile_pool(name="Lp", bufs=2, space="PSUM") as Lp,
            tc.tile_pool(name="Tp", bufs=2, space="PSUM") as Tp,
            tc.tile_pool(name="Cp", bufs=2, space="PSUM") as Cp,
        ):
            for qt in range(4):
                for h in range(8):
                    ht = h // 2
                    hp = (h % 2) * 64
                    nch = (qt + 1) if causal else 4   # live 512-chunks
                    live = nch * 512
                    P_t = Pp.tile([128, S], FP, tag="p", name="P_t")
                    s_parts = []
                    # logits psum in halves of up to 1024 cols
                    for h0 in range(0, nch, 2):
                        hw = min(2, nch - h0) * 512
                        L_t = Lp.tile([128, 1024], FP, tag="L", name="L_t")
                        for sc in range(h0, min(h0 + 2, nch)):
                            sl = slice((sc - h0) * 512, (sc - h0 + 1) * 512)
                            nc.tensor.matmul(
                                L_t[:, sl],
                                QT[ht][hp:hp + 64, qt * 128:(qt + 1) * 128],
                                KT[ht][hp:hp + 64, sc * 512:(sc + 1) * 512],
                                start=True,
                                stop=(causal and sc != qt))
                            if causal:
                                if sc == qt:
                                    nc.tensor.matmul(
                                        L_t[:, sl], i80k, MB[qt],
                                        start=False, stop=True)
                            else:
                                nc.tensor.matmul(
                                    L_t[:, sl], i80k,
                                    MB[qt][:, sc * 512:(sc + 1) * 512],
                                    start=False, stop=True)
                        sh = sp.tile([128, 1], FP, tag="sh", name="sh")
                        nc.scalar.activation(
                            out=P_t[:, h0 * 512:h0 * 512 + hw],
                            in_=L_t[:, :hw],
                            func=mybir.ActivationFunctionType.Exp,
                            scale=0.125, accum_out=sh)
                        s_parts.append(sh)
                    rs = sp.tile([128, 1], FP, tag="rs", name="rs")
                    if len(s_parts) == 1:
                        nc.vector.reciprocal(rs, s_parts[0])
                    else:
                        nc.vector.tensor_add(rs, s_parts[0], s_parts[1])
                        nc.vector.reciprocal(rs, rs)
                    nc.vector.tensor_scalar_mul(
                        P_t[:, :live], P_t[:, :live], rs)
                    nc.sync.dma_start(
                        out=d["attn_o"][h, qt * 128:(qt + 1) * 128, :live],
                        in_=P_t[:, :live])
                    if causal and live < S:
                        nc.sync.dma_start(
                            out=d["attn_o"][h, qt * 128:(qt + 1) * 128,
                                            live:],
                            in_=zerot[:, :S - live])
                    # transpose live 128-blocks -> PT
                    PT_t = PTp.tile([128, S], FP, tag="pt", name="PT_t")
                    nkt = nch * 4
                    for jg in range((nkt + 3) // 4):
                        T_t = Tp.tile([128, 512], FP, tag="tp", name="T_t")
                        n_in_g = min(4, nkt - jg * 4)
                        for jj in range(n_in_g):
                            j = jg * 4 + jj
                            nc.tensor.transpose(
                                T_t[:, jj * 128:(jj + 1) * 128],
                                P_t[:, j * 128:(j + 1) * 128], ident)
                        dst = PT_t[:, jg * 512:jg * 512 + n_in_g * 128]
                        if jg % 2 == 0:
                            nc.vector.tensor_copy(dst, T_t[:, :n_in_g * 128])
                        else:
                            nc.scalar.copy(dst, T_t[:, :n_in_g * 128])
                    # PV: ctx^T[64, 128q] over live k-tiles
                    ct = Cp.tile([64, 128], FP, tag="ctx", name="ct")
                    for j in range(nkt):
                        nc.tensor.matmul(
                            ct, Vt[j][:, h * 64:(h + 1) * 64],
                            PT_t[:, j * 128:(j + 1) * 128],
                            start=(j == 0), stop=(j == nkt - 1))
                    nc.vector.tensor_copy(
                        CT[ht][hp:hp + 64, qt * 128:(qt + 1) * 128], ct)

                # ---- output projection + residual + LayerNorm for qt ----
                O_ps = Tp.tile([128, 512], FP, tag="tp", name="O_ps")
                for din in range(4):
                    nc.tensor.matmul(
                        O_ps, CT[din][:, qt * 128:(qt + 1) * 128],
                        wo_s[din], start=(din == 0), stop=(din == 3))
                O_sb = Osb.tile([128, D], FP, tag="osb", name="O_sb")
                nc.vector.tensor_add(O_sb, O_ps, QN[qt])
                stats = sp.tile([128, 6], FP, tag="st", name="stats")
                mv = sp.tile([128, 2], FP, tag="mv", name="mv")
                nc.vector.bn_stats(stats, O_sb)
                nc.vector.bn_aggr(mv, stats)
                rstd = sp.tile([128, 1], FP, tag="rstd", name="rstd")
                nc.scalar.activation(
                    out=rstd, in_=mv[:, 1:2],
                    func=mybir.ActivationFunctionType.Sqrt, bias=eps_t)
                nc.vector.reciprocal(rstd, rstd)
                nc.vector.tensor_scalar(
                    out=O_sb, in0=O_sb, scalar1=mv[:, 0:1], scalar2=rstd,
                    op0=mybir.AluOpType.subtract, op1=mybir.AluOpType.mult)
                nc.vector.tensor_mul(O_sb, O_sb, ga_b)
                nc.vector.tensor_add(O_sb, O_sb, be_b)
                nc.sync.dma_start(
                    out=d["out_o"][qt * 128:(qt + 1) * 128, :], in_=O_sb)


_NC_CACHE = {}


def _get_nc(causal):
    if causal not in _NC_CACHE:
        _NC_CACHE[causal] = _build_nc(causal)
    return _NC_CACHE[causal]


def _qtiles(j):
    return sorted([j, 7 - j, 8 + j, 15 - j])


def kernel(query, key, value, mask, wq, bq, wk, bk, wv, bv, wo, bo,
           gamma, beta):
    query = np.ascontiguousarray(np.asarray(query, np.float32))
    key = np.ascontiguousarray(np.asarray(key, np.float32))
    value = np.ascontiguousarray(np.asarray(value, np.float32))
    mask = np.ascontiguousarray(np.asarray(mask, np.float32))
    wq = np.ascontiguousarray(np.asarray(wq, np.float32))
    wk = np.ascontiguousarray(np.asarray(wk, np.float32))
    wv = np.ascontiguousarray(np.asarray(wv, np.float32))
    wo = np.ascontiguousarray(np.asarray(wo, np.float32))
    bq = np.asarray(bq, np.float32)
    bk = np.asarray(bk, np.float32)
    bv = np.asarray(bv, np.float32)
    bo = np.asarray(bo, np.float32)
    gamma = np.asarray(gamma, np.float32)
    beta = np.asarray(beta, np.float32)

    causal_ref = np.triu(np.ones((S, S), np.float32), k=1)
    causal = all(np.array_equal(mask[b, 0], causal_ref) for b in range(B))

    nc = _get_nc(causal)
    in_maps = []
    for c in range(NCORES):
        b = c // 4
        j = c % 4
        tiles = _qtiles(j)
        qrows = np.concatenate(
            [np.arange(g * 128, (g + 1) * 128) for g in tiles])
        qs = query[b][qrows]
        if causal:
            mk = np.stack([
                mask[b, 0, tiles[t] * 128:(tiles[t] + 1) * 128,
                     t * 512:(t + 1) * 512]
                for t in range(NQT)]).astype(ml_dtypes.bfloat16)
        else:
            mk = mask[b, 0][qrows].astype(ml_dtypes.bfloat16)
        in_maps.append({
            "qT": np.ascontiguousarray(qs.T),
            "qn": np.ascontiguousarray(qs + bo[None, :]),
            "kT": np.ascontiguousarray(key[b].T),
            "vT": np.ascontiguousarray(value[b].T),
            "mk": np.ascontiguousarray(mk),
            "wq": wq, "wk": wk, "wv": wv, "wo": wo,
            "bq": bq, "bk": bk, "bv": bv,
            "ga": gamma, "be": beta,
        })

    r = run_bass_kernel_spmd(nc, in_maps, core_ids=list(range(NCORES)),
                             trace=TRACE)
    if TRACE:
        kernel.last_results = r

    out = np.empty((B, S, D), np.float32)
    attn = np.empty((B, H, S, S), np.float32)
    for c in range(NCORES):
        b = c // 4
        tiles = _qtiles(c % 4)
        for t, g in enumerate(tiles):
            out[b, g * 128:(g + 1) * 128, :] = \
                r.results[c]["out_o"][t * 128:(t + 1) * 128]
            attn[b, :, g * 128:(g + 1) * 128, :] = \
                r.results[c]["attn_o"][:, t * 128:(t + 1) * 128, :]
    return out, attn
